# revision 1
# baseline (speedup 1.0000x reference)
"""IsoMax pairwise-distance kernel for 8 TRN2 NeuronCores.

Math:  out[b,m] = -|s| * sqrt(max(||xn_b||^2 + ||pn_m||^2 - 2*xn_b.pn_m, 0))
with xn/pn L2-normalized rows of x [4096,2048] and prototypes [12893,2048].
Since xn,pn are unit vectors this is -|s|*sqrt(2 - 2*cos). We compute
G = x_bf16 @ pn_bf16^T on the PE (pn pre-normalized, x raw) and fuse the
epilogue into one ACT pass: sqrt(scale_vec[b]*G + 2s^2), scale_vec = -2s^2/||x_b||,
then a DVE negate.

Sharding: prototypes split across the 8 cores (output columns), x replicated.
M=12893 padded to 13312 = 8*1664 (zero rows -> harmless, sliced off on host).
"""

import os
import sys

sys.path.insert(0, "/opt/trn_rl_repo")

import numpy as np

B = 4096
D = 2048
M_FULL = 12893
N_CORES = 8
MC = 1664  # per-core prototype rows (13*128); 8*1664 = 13312 >= 12893
P = 128
KT = D // P  # 16 contraction chunks
MT = MC // P  # 13 m-tiles per core
BT = B // P  # 32 b-tiles

_cache = {}


def _build(s_abs: float, b_rows: int = B, mc: int = MC, repeat: int = 1,
           mm_order: str = "k_major"):
    import concourse.bass as bass  # noqa: F401
    import concourse.mybir as mybir
    import concourse.tile as tile
    from concourse import bacc
    from contextlib import ExitStack

    f32 = mybir.dt.float32
    bf16 = mybir.dt.bfloat16
    AF = mybir.ActivationFunctionType
    kt = D // P
    mt_n = mc // P
    bt_n = b_rows // P
    two_s2 = 2.0 * s_abs * s_abs

    # psum chunks over mc columns (<=512 wide, multiples of 128)
    chunks = []
    off = 0
    while off < mc:
        w = min(512, mc - off)
        chunks.append((off, w))
        off += w

    nc = bacc.Bacc(None, target_bir_lowering=False)
    x_d = nc.dram_tensor("x", [b_rows, D], f32, kind="ExternalInput")
    p_d = nc.dram_tensor("p", [mc, D], f32, kind="ExternalInput")
    o_d = nc.dram_tensor("o", [b_rows, mc], f32, kind="ExternalOutput")

    with ExitStack() as ctx:
        tc = ctx.enter_context(tile.TileContext(nc))
        persist = ctx.enter_context(tc.tile_pool(name="persist", bufs=1))
        ppool = ctx.enter_context(tc.tile_pool(name="ppool", bufs=3))
        sq = ctx.enter_context(tc.tile_pool(name="sq", bufs=2))
        small = ctx.enter_context(tc.tile_pool(name="small", bufs=6))
        xpool = ctx.enter_context(tc.tile_pool(name="xpool", bufs=3))
        xtpool = ctx.enter_context(tc.tile_pool(name="xtpool", bufs=3))
        opool = ctx.enter_context(tc.tile_pool(name="opool", bufs=3))
        psum = ctx.enter_context(tc.tile_pool(name="psum", bufs=8, space="PSUM"))

        # prototypes, normalized, bf16, transposed: [d_inner, m_tile, k, m_inner]
        pT = persist.tile([P, mt_n, kt, P], bf16)

        # bias tiles for ACT (const-AP db has no arbitrary constants)
        tiny_b = persist.tile([P, 1], f32, tag="tiny_b")
        nc.vector.memset(tiny_b, 1e-30)
        two_s2_b = persist.tile([P, 1], f32, tag="two_s2_b")
        nc.vector.memset(two_s2_b, two_s2)

        for mt in range(mt_n):
            p_f = ppool.tile([P, D], f32, tag="p_f")
            nc.sync.dma_start(p_f, p_d[mt * P : (mt + 1) * P, :])
            psq = sq.tile([P, D], f32, tag="sq")
            ssp = small.tile([P, 1], f32, tag="ss")
            # ssp = sum_d p^2 (Square+Sqrt share one ACT table set)
            nc.scalar.activation(psq, p_f, AF.Square, accum_out=ssp)
            pnorm = small.tile([P, 1], f32, tag="nrm")
            nc.scalar.activation(pnorm, ssp, AF.Sqrt, bias=tiny_b)
            rp = small.tile([P, 1], f32, tag="rp")
            nc.vector.reciprocal(rp, pnorm)
            pn = ppool.tile([P, D], bf16, tag="pn")
            nc.vector.tensor_scalar_mul(pn, p_f, rp)
            nc.sync.dma_start(pT[:, mt], pn, transpose=True)

        for bt_r in range(bt_n * repeat):
            bt = bt_r % bt_n
            x_bf = xpool.tile([P, D], bf16, tag="x_bf")
            # SWDGE dma casts f32 -> bf16 in flight
            nc.gpsimd.dma_start(x_bf, x_d[bt * P : (bt + 1) * P, :])
            xsq = sq.tile([P, D], f32, tag="sq")
            ssx = small.tile([P, 1], f32, tag="ss")
            nc.scalar.activation(xsq, x_bf, AF.Square, accum_out=ssx)
            xnorm = small.tile([P, 1], f32, tag="nrm")
            nc.scalar.activation(xnorm, ssx, AF.Sqrt, bias=tiny_b)
            rx = small.tile([P, 1], f32, tag="rx")
            nc.vector.reciprocal(rx, xnorm)
            svec = small.tile([P, 1], f32, tag="svec")
            nc.vector.tensor_scalar_mul(svec, rx, -two_s2)

            xT = xtpool.tile([P, kt, P], bf16, tag="xT")
            nc.sync.dma_start(xT, x_bf, transpose=True)

            pts = [
                psum.tile([P, 512], f32, tag="ps", name=f"ps_{ci}")[:, :w]
                for ci, (_o, w) in enumerate(chunks)
            ]
            if mm_order == "k_major":
                mm_iter = [(k, ci) for k in range(kt) for ci in range(len(chunks))]
            else:  # chunk_major: finish one PSUM bank's accumulation group first
                mm_iter = [(k, ci) for ci in range(len(chunks)) for k in range(kt)]
            for k, ci in mm_iter:
                coff, w = chunks[ci]
                mt0 = coff // P
                nt = w // P
                nc.tensor.matmul(
                    pts[ci],
                    xT[:, k, :],
                    pT[:, mt0 : mt0 + nt, k, :],
                    start=(k == 0),
                    stop=(k == kt - 1),
                )
            t_sb = opool.tile([P, mc], f32, tag="t_sb")
            for ci, (coff, w) in enumerate(chunks):
                # sqrt(-2s^2/||x|| * G + 2s^2) = s*sqrt(2 - 2*cos)
                nc.scalar.activation(
                    t_sb[:, coff : coff + w], pts[ci], AF.Sqrt,
                    bias=two_s2_b, scale=svec,
                )
            nc.vector.tensor_scalar_mul(t_sb, t_sb, -1.0)
            nc.sync.dma_start(o_d[bt * P : (bt + 1) * P, :], t_sb)

    nc.compile()
    return nc


LAST_RESULT = None


def _run(nc, in_maps, core_ids):
    from concourse import bass_utils

    global LAST_RESULT
    trace = bool(int(os.environ.get("ISOMAX_TRACE", "0")))
    LAST_RESULT = bass_utils.run_bass_kernel_spmd(
        nc, in_maps, core_ids=core_ids, trace=trace
    )
    return LAST_RESULT.results


def kernel(x, prototypes, distance_scale):
    x = np.ascontiguousarray(np.asarray(x, dtype=np.float32))
    p = np.asarray(prototypes, dtype=np.float32)
    s_abs = float(abs(np.asarray(distance_scale).reshape(-1)[0].item()))
    m, d = p.shape
    assert (m, d) == (M_FULL, D) and x.shape == (B, D)

    key = ("full", s_abs)
    if key not in _cache:
        _cache[key] = _build(s_abs)
    nc = _cache[key]

    p_pad = np.zeros((N_CORES * MC, D), np.float32)
    p_pad[:m] = p
    in_maps = [
        {"x": x, "p": np.ascontiguousarray(p_pad[i * MC : (i + 1) * MC])}
        for i in range(N_CORES)
    ]
    results = _run(nc, in_maps, list(range(N_CORES)))
    out = np.concatenate([results[i]["o"] for i in range(N_CORES)], axis=1)
    return np.ascontiguousarray(out[:, :m]).astype(np.float32)



# revision 2
# speedup vs baseline: 1.0115x; 1.0115x over previous
"""IsoMax pairwise-distance kernel for 8 TRN2 NeuronCores (fp8 DoubleRow, v5).

Math:  out[b,m] = -|s| * sqrt(max(||xn_b||^2 + ||pn_m||^2 - 2*xn_b.pn_m, 0))
with xn/pn L2-normalized rows of x [4096,2048] and prototypes [12893,2048].
Since xn,pn are unit vectors this is -|s|*sqrt(2 - 2*cos). BOTH operands are
L2-normalized on device and scaled by 32 before fp8e4 quantization (their
~N(0, 1/2048) entries land in fp8's normal range), so the PE computes
G = (32*xn)_fp8 @ (32*pn)_fp8^T = 1024*cos with perf_mode=DoubleRow
(2 fp8 weights/cell -> 2 contraction rows per pass). The epilogue is one
ACT pass with a CONSTANT scale: sqrt(-2s^2/1024 * G + 2s^2) = |s|*sqrt(2-2cos),
then a DVE negate; output stored bf16 and upcast on host.

Engine queues execute in program order, so the emission order IS the
schedule:
  - prologue: prototype m-tiles for the 512-wide psum chunks and the first
    PRE x b-tiles, normalized/transposed/quantized (also building a
    persistent transposed-fp8 copy of all of x, 64KB/partition);
  - main loop, one iteration per b-tile: prep of b-tile bt+PRE (keeps every
    queue PRE iterations ahead of the PE), then 24 back-to-back matmuls
    accumulating all three 512-wide chunks into ONE 3-bank psum tile,
    one epilogue ACT over [128,1536], one DVE negate, one store;
  - the last 128-wide chunk would be LDWEIGHTS-bound x-stationary (213ns
    weight load vs 56ns matmul), so it instead makes PROTOTYPES stationary
    and streams 512 b-columns per matmul into [m, b] psum tiles, two
    b-groups per round on the two spare psum banks; outputs are transposed
    back via the DMA xbar before the store. The constant epilogue scale is
    what makes this transposed layout legal.
Inputs are staged to bf16 on the host, halving input DMA traffic. DMA work
is spread over three queues: SWDGE does loads, the sync HWDGE ring does
transposes + stores (ACT-ring stores head-of-line block the epilogue ACTs).

Sharding: prototypes split across the 8 cores (output columns), x replicated.
M=12893 padded to 13312 = 8*1664 (zero rows -> harmless, sliced off on host).
"""

import os
import sys

sys.path.insert(0, "/opt/trn_rl_repo")

import numpy as np

B = 4096
D = 2048
M_FULL = 12893
N_CORES = 8
MC = 1664  # per-core prototype rows (13*128); 8*1664 = 13312 >= 12893
P = 128
KT = D // P  # 16 contraction chunks
BT = B // P  # 32 b-tiles
PSCALE = 32.0  # xn/pn are quantized as 32*(unit vector) for fp8e4 range

_cache = {}


def _build(s_abs: float, b_rows: int = B, mc: int = MC, repeat: int = 1):
    import concourse.bass as bass  # noqa: F401
    import concourse.mybir as mybir
    import concourse.tile as tile
    from concourse import bacc
    from contextlib import ExitStack

    f32 = mybir.dt.float32
    bf16 = mybir.dt.bfloat16
    fp8 = mybir.dt.float8e4
    AF = mybir.ActivationFunctionType
    DR = mybir.MatmulPerfMode.DoubleRow
    kt = D // P
    kp_n = kt // 2
    mt_n = mc // P
    bt_n = b_rows // P
    two_s2 = 2.0 * s_abs * s_abs
    # G = 1024*cos -> scale inside the epilogue sqrt is constant
    g_scale = -two_s2 / (PSCALE * PSCALE)

    # psum chunks over mc columns; 512-wide so chunk boundaries align with
    # the 128-row prototype m-tiles (1664 = 512+512+512+128)
    chunks = []
    off = 0
    while off < mc:
        w = min(512, mc - off)
        chunks.append((off, w))
        off += w
    n_ck = len(chunks)
    # the last chunk runs p-stationary iff it is 128 wide (one m-tile) and
    # b splits into 512-col groups
    p_stat_last = chunks[-1][1] == P and b_rows % 512 == 0
    n_norm_ck = n_ck - 1 if p_stat_last else n_ck
    norm_w = chunks[n_norm_ck - 1][0] + chunks[n_norm_ck - 1][1]

    nc = bacc.Bacc(None, target_bir_lowering=False)
    x_d = nc.dram_tensor("x", [b_rows, D], bf16, kind="ExternalInput")
    p_d = nc.dram_tensor("p", [mc, D], bf16, kind="ExternalInput")
    o_d = nc.dram_tensor("o", [b_rows, mc], bf16, kind="ExternalOutput")

    with ExitStack() as ctx:
        tc = ctx.enter_context(tile.TileContext(nc))
        persist = ctx.enter_context(tc.tile_pool(name="persist", bufs=1))
        ppool = ctx.enter_context(tc.tile_pool(name="ppool", bufs=6))
        ptpool = ctx.enter_context(tc.tile_pool(name="ptpool", bufs=4))
        sq = ctx.enter_context(tc.tile_pool(name="sq", bufs=2))
        small = ctx.enter_context(tc.tile_pool(name="small", bufs=8))
        opool = ctx.enter_context(tc.tile_pool(name="opool", bufs=4))
        psum = ctx.enter_context(tc.tile_pool(name="psum", bufs=2, space="PSUM"))

        # prototypes, normalized*32, fp8, one tile per psum chunk:
        # [d_inner, k_chunk, m_cols_of_chunk]
        pT8s = [
            persist.tile([P, kt, w], fp8, tag=f"pT8_{ci}", name=f"pT8_{ci}")
            for ci, (_o, w) in enumerate(chunks)
        ]
        # all of x, normalized*32, transposed, fp8: [d_inner, k_chunk, b]
        xT8all = persist.tile([P, kt, b_rows], fp8, tag="xT8all")

        # bias tiles for ACT (const-AP db has no arbitrary constants)
        tiny_b = persist.tile([P, 1], f32, tag="tiny_b")
        nc.vector.memset(tiny_b, 1e-30)
        two_s2_b = persist.tile([P, 1], f32, tag="two_s2_b")
        nc.vector.memset(two_s2_b, two_s2)

        def rows_prep(src_d, row0, dst8, dcol0):
            # load 128 rows, L2-normalize, scale by 32, transpose, cast to
            # fp8 into dst8[:, :, dcol0:dcol0+128]
            r_f = ppool.tile([P, D], bf16, tag="r_f", name="r_f")
            # SWDGE: keeps the sync HWDGE ring free for transposes/stores
            nc.gpsimd.dma_start(r_f, src_d[row0 : row0 + P, :])
            rsq = sq.tile([P, D], bf16, tag="sq", name="rsq")
            ss = small.tile([P, 1], f32, tag="ss", name="ss")
            nc.scalar.activation(rsq, r_f, AF.Square, accum_out=ss)
            # ||r||/32 via sqrt(ss/1024)
            rnorm = small.tile([P, 1], f32, tag="nrm", name="rnorm")
            nc.scalar.activation(
                rnorm, ss, AF.Sqrt, bias=tiny_b, scale=1.0 / (PSCALE * PSCALE)
            )
            rr = small.tile([P, 1], f32, tag="rp", name="rr")
            nc.vector.reciprocal(rr, rnorm)  # 32/||r||
            rn = ppool.tile([P, D], bf16, tag="rn", name="rn")
            nc.vector.tensor_scalar_mul(rn, r_f, rr)
            rTb = ptpool.tile([P, kt, P], bf16, tag="pTb", name="rTb")
            nc.sync.dma_start(rTb, rn, transpose=True)
            # cast into the fp8 matmul layout (dim1 stride = dst width)
            nc.vector.tensor_copy(dst8[:, :, dcol0 : dcol0 + P], rTb)

        def p_prep(mt):
            ci = min(mt // 4, n_ck - 1)
            rows_prep(p_d, mt * P, pT8s[ci], (mt - 4 * ci) * P)

        def x_prep(bt):
            rows_prep(x_d, bt * P, xT8all, bt * P)

        PRE = 6
        # chunk 0's prototype m-tiles, then the first x b-tiles, then the
        # remaining prototype m-tiles: the first matmuls can start while
        # the rest of the prototype table is still being prepared
        for mt in range(min(4, mt_n)):
            p_prep(mt)
        for bt in range(min(PRE, bt_n)):
            x_prep(bt)
        for mt in range(min(4, mt_n), mt_n):
            p_prep(mt)

        for rep in range(repeat):
            for bt in range(bt_n):
                if rep == 0 and bt + PRE < bt_n:
                    x_prep(bt + PRE)
                pt = psum.tile([P, norm_w], f32, tag="psA", name="pt")
                for ci in range(n_norm_ck):
                    coff, w = chunks[ci]
                    for kp in range(kp_n):
                        nc.tensor.matmul(
                            pt[:, coff : coff + w],
                            xT8all[:, 2 * kp : 2 * kp + 2, bt * P : (bt + 1) * P],
                            pT8s[ci][:, 2 * kp : 2 * kp + 2, :],
                            start=(kp == 0),
                            stop=(kp == kp_n - 1),
                            perf_mode=DR,
                        )
                t_c = opool.tile([P, norm_w], bf16, tag="t_c", name="t_c")
                # sqrt(-2s^2/1024 * G + 2s^2) = s*sqrt(2 - 2*cos)
                nc.scalar.activation(
                    t_c, pt, AF.Sqrt, bias=two_s2_b, scale=g_scale
                )
                nc.vector.tensor_scalar_mul(t_c, t_c, -1.0)
                nc.sync.dma_start(
                    o_d[bt * P : (bt + 1) * P, 0:norm_w], t_c
                )

            if p_stat_last:
                # last chunk: prototypes stationary, 512 b-cols moving;
                # psum holds [m, b] tiles on the 2 spare banks, 2 b-groups
                # per round
                coff, w = chunks[-1]
                bg_n = b_rows // 512
                for bg0 in range(0, bg_n, 2):
                    bgs = range(bg0, min(bg0 + 2, bg_n))
                    pts3 = {
                        bg: psum.tile([P, 512], f32, tag="ps3", name=f"ps3_{bg}")
                        for bg in bgs
                    }
                    for kp in range(kp_n):
                        for bg in bgs:
                            nc.tensor.matmul(
                                pts3[bg],
                                pT8s[-1][:, 2 * kp : 2 * kp + 2, :],
                                xT8all[
                                    :,
                                    2 * kp : 2 * kp + 2,
                                    bg * 512 : (bg + 1) * 512,
                                ],
                                start=(kp == 0),
                                stop=(kp == kp_n - 1),
                                perf_mode=DR,
                            )
                    for bg in bgs:
                        t3 = opool.tile([P, 512], bf16, tag="t3", name="t3")
                        nc.scalar.activation(
                            t3, pts3[bg], AF.Sqrt, bias=two_s2_b, scale=g_scale
                        )
                        nc.vector.tensor_scalar_mul(t3, t3, -1.0)
                        # transpose [m, 512b] -> [b_inner, 4 b_outer, m]
                        t3T = ptpool.tile([P, 4, P], bf16, tag="t3T", name="t3T")
                        nc.sync.dma_start(t3T, t3, transpose=True)
                        for bo in range(4):
                            b0 = bg * 512 + bo * P
                            nc.sync.dma_start(
                                o_d[b0 : b0 + P, coff : coff + w], t3T[:, bo, :]
                            )

    nc.compile()
    return nc


LAST_RESULT = None


def _run(nc, in_maps, core_ids):
    from concourse import bass_utils

    global LAST_RESULT
    trace = bool(int(os.environ.get("ISOMAX_TRACE", "0")))
    LAST_RESULT = bass_utils.run_bass_kernel_spmd(
        nc, in_maps, core_ids=core_ids, trace=trace
    )
    return LAST_RESULT.results


def kernel(x, prototypes, distance_scale):
    import ml_dtypes

    bf16 = ml_dtypes.bfloat16
    x = np.asarray(x, dtype=np.float32)
    p = np.asarray(prototypes, dtype=np.float32)
    s_abs = float(abs(np.asarray(distance_scale).reshape(-1)[0].item()))
    m, d = p.shape
    assert (m, d) == (M_FULL, D) and x.shape == (B, D)

    key = ("full", s_abs)
    if key not in _cache:
        _cache[key] = _build(s_abs)
    nc = _cache[key]

    x_bf = np.ascontiguousarray(x.astype(bf16))
    p_pad = np.zeros((N_CORES * MC, D), bf16)
    p_pad[:m] = p.astype(bf16)
    in_maps = [
        {"x": x_bf, "p": np.ascontiguousarray(p_pad[i * MC : (i + 1) * MC])}
        for i in range(N_CORES)
    ]
    results = _run(nc, in_maps, list(range(N_CORES)))
    out = np.concatenate(
        [results[i]["o"].astype(np.float32) for i in range(N_CORES)], axis=1
    )
    return np.ascontiguousarray(out[:, :m])


# revision 3
# speedup vs baseline: 1.2824x; 1.2678x over previous
"""IsoMax pairwise-distance kernel for 8 TRN2 NeuronCores (fp8 DoubleRow, v6).

Math:  out[b,m] = -|s| * sqrt(max(||xn_b||^2 + ||pn_m||^2 - 2*xn_b.pn_m, 0))
with xn/pn L2-normalized rows of x [4096,2048] and prototypes [12893,2048].
Since xn,pn are unit vectors this is -|s|*sqrt(2 - 2*cos(x_b, p_m)).

The PE computes Graw = x_fp8 @ p_fp8^T (RAW values quantized to fp8e4 —
they are O(1) so no scaling/subnormal issues) with perf_mode=DoubleRow.
Normalization happens entirely in the epilogue:
    Gn = Graw * RP          (DVE; RP = 1/||p_m|| broadcast along partitions)
    t  = sqrt(svec_b * Gn + 2s^2)   (ACT; svec_b = -2s^2/||x_b|| per-partition)
    out = -t                (DVE negate), stored bf16, upcast on host.

Earlier revisions transposed x/p on device; the DMA-xbar transposes
serialize (~5-7us round-trip each) and dominated the span. Here the HOST
stages both operands twice: d-major (xT [D,B], pT [D,MC] — fed straight to
fp8 casts for the matmul, one bulk DMA + one DVE cast per 512-column group)
and row-major (only read by the norm pipeline: square+accum -> sqrt ->
reciprocal, all off the matmul critical path). RP (a per-OUTPUT-COLUMN
vector) is produced by writing the 13 per-m-tile reciprocal-norm vectors to
a DRAM scratch buffer and re-reading them with a 0-stride partition
broadcast AP on the same SWDGE queue (the writes sit hundreds of queue
slots ahead of the read).

The last 128-wide psum chunk would be LDWEIGHTS-bound x-stationary (213ns
weight load vs 56ns matmul), so it makes PROTOTYPES stationary and streams
512 b-columns per matmul into [m, b] psum tiles; its per-m factor rides the
ACT scale AP and the per-b factor (RX = 1/||x_b||) is broadcast like RP.
The [m, b] outputs are stored via a DMA xbar transpose (only 8 of them).

Sharding: prototypes split across the 8 cores (output columns), x replicated.
M=12893 padded to 13312 = 8*1664 (zero rows -> harmless, sliced off on host).
"""

import os
import sys

sys.path.insert(0, "/opt/trn_rl_repo")

import numpy as np

B = 4096
D = 2048
M_FULL = 12893
N_CORES = 8
MC = 1664  # per-core prototype rows (13*128); 8*1664 = 13312 >= 12893
P = 128
KT = D // P  # 16 contraction chunks
BT = B // P  # 32 b-tiles

_cache = {}


def _build(s_abs: float, b_rows: int = B, mc: int = MC, repeat: int = 1):
    import concourse.bass as bass  # noqa: F401
    import concourse.mybir as mybir
    import concourse.tile as tile
    from concourse import bacc
    from contextlib import ExitStack

    f32 = mybir.dt.float32
    bf16 = mybir.dt.bfloat16
    fp8 = mybir.dt.float8e4
    AF = mybir.ActivationFunctionType
    DR = mybir.MatmulPerfMode.DoubleRow
    kt = D // P
    kp_n = kt // 2
    mt_n = mc // P
    bt_n = b_rows // P
    two_s2 = 2.0 * s_abs * s_abs

    # psum chunks over mc columns; 512-wide, aligned with prototype m-tiles
    chunks = []
    off = 0
    while off < mc:
        w = min(512, mc - off)
        chunks.append((off, w))
        off += w
    n_ck = len(chunks)
    p_stat_last = chunks[-1][1] == P and b_rows % 512 == 0
    n_norm_ck = n_ck - 1 if p_stat_last else n_ck
    norm_w = chunks[n_norm_ck - 1][0] + chunks[n_norm_ck - 1][1]
    bg_n = (b_rows + 511) // 512

    nc = bacc.Bacc(None, target_bir_lowering=False)
    x_d = nc.dram_tensor("x", [b_rows, D], bf16, kind="ExternalInput")
    p_d = nc.dram_tensor("p", [mc, D], bf16, kind="ExternalInput")
    xt_d = nc.dram_tensor("xt", [D, b_rows], bf16, kind="ExternalInput")
    pt_d = nc.dram_tensor("pt", [D, mc], bf16, kind="ExternalInput")
    o_d = nc.dram_tensor("o", [b_rows, mc], bf16, kind="ExternalOutput")
    rp_d = nc.dram_tensor("rp_scratch", [mc], f32, kind="Internal")
    rx_d = nc.dram_tensor("rx_scratch", [b_rows], f32, kind="Internal")

    with ExitStack() as ctx:
        tc = ctx.enter_context(tile.TileContext(nc))
        persist = ctx.enter_context(tc.tile_pool(name="persist", bufs=1))
        dpool = ctx.enter_context(tc.tile_pool(name="dpool", bufs=2))
        npool = ctx.enter_context(tc.tile_pool(name="npool", bufs=4))
        sq = ctx.enter_context(tc.tile_pool(name="sq", bufs=2))
        small = ctx.enter_context(tc.tile_pool(name="small", bufs=8))
        opool = ctx.enter_context(tc.tile_pool(name="opool", bufs=4))
        psum = ctx.enter_context(tc.tile_pool(name="psum", bufs=2, space="PSUM"))

        # fp8 d-major operands: [d_inner, k_chunk, col]
        pT8s = [
            persist.tile([P, kt, w], fp8, tag=f"pT8_{ci}", name=f"pT8_{ci}")
            for ci, (_o, w) in enumerate(chunks)
        ]
        xT8all = persist.tile([P, kt, b_rows], fp8, tag="xT8all")
        # epilogue factors
        svec_all = persist.tile([P, bt_n], f32, tag="svec_all")
        RP = persist.tile([P, norm_w], bf16, tag="RP")  # 1/||p_m|| bcast
        if p_stat_last:
            RX = persist.tile([P, b_rows], bf16, tag="RX")  # 1/||x_b|| bcast
            rp12s = persist.tile([P, 1], f32, tag="rp12s")  # -2s^2/||p_m||

        tiny_b = persist.tile([P, 1], f32, tag="tiny_b")
        nc.vector.memset(tiny_b, 1e-30)
        two_s2_b = persist.tile([P, 1], f32, tag="two_s2_b")
        nc.vector.memset(two_s2_b, two_s2)

        def dmaj_load(src_t, c0, w, dst8, dc0):
            # bulk d-major block [128, kt, w] -> fp8 into dst8[:, :, dc0:+w]
            blk = dpool.tile([P, kt, 512], bf16, tag="blk", name="blk")[:, :, :w]
            src = src_t[:, c0 : c0 + w].rearrange(
                "(k p) c -> p k c", p=P
            )
            nc.gpsimd.dma_start(blk, src)
            nc.vector.tensor_copy(dst8[:, :, dc0 : dc0 + w], blk)

        def norm_prep(src_d, row0, sink):
            # row-major 128 rows -> 1/||row|| [128,1], handed to sink
            r_f = npool.tile([P, D], bf16, tag="r_f", name="r_f")
            nc.sync.dma_start(r_f, src_d[row0 : row0 + P, :])
            rsq = sq.tile([P, D], bf16, tag="sq", name="rsq")
            ss = small.tile([P, 1], f32, tag="ss", name="ss")
            nc.scalar.activation(rsq, r_f, AF.Square, accum_out=ss)
            rnorm = small.tile([P, 1], f32, tag="nrm", name="rnorm")
            nc.scalar.activation(rnorm, ss, AF.Sqrt, bias=tiny_b)
            rr = small.tile([P, 1], f32, tag="rr", name="rr")
            nc.vector.reciprocal(rr, rnorm)
            sink(rr)

        def p_norm_sink(mt):
            def sink(rr):
                # stash 1/||p|| to DRAM for the RP broadcast read
                nc.gpsimd.dma_start(rp_d[mt * P : (mt + 1) * P], rr)
                if p_stat_last and mt == mt_n - 1:
                    nc.vector.tensor_scalar_mul(rp12s, rr, -two_s2)
            return sink

        def x_norm_sink(bt):
            def sink(rr):
                nc.vector.tensor_scalar_mul(
                    svec_all[:, bt : bt + 1], rr, -two_s2
                )
                if p_stat_last:
                    nc.gpsimd.dma_start(rx_d[bt * P : (bt + 1) * P], rr)
            return sink

        def mm_epi(bt):
            pt = psum.tile([P, norm_w], f32, tag="psA", name="pt")
            for ci in range(n_norm_ck):
                coff, w = chunks[ci]
                for kp in range(kp_n):
                    nc.tensor.matmul(
                        pt[:, coff : coff + w],
                        xT8all[:, 2 * kp : 2 * kp + 2, bt * P : (bt + 1) * P],
                        pT8s[ci][:, 2 * kp : 2 * kp + 2, :],
                        start=(kp == 0),
                        stop=(kp == kp_n - 1),
                        perf_mode=DR,
                    )
            nc.vector.tensor_mul(pt, pt, RP)  # Graw -> Graw/||p_m||
            t_c = opool.tile([P, norm_w], bf16, tag="t_c", name="t_c")
            nc.scalar.activation(
                t_c, pt, AF.Sqrt, bias=two_s2_b,
                scale=svec_all[:, bt : bt + 1],
            )
            nc.vector.tensor_scalar_mul(t_c, t_c, -1.0)
            nc.sync.dma_start(o_d[bt * P : (bt + 1) * P, 0:norm_w], t_c)

        PRE = 6
        # prologue: p + first x b-group d-major; all p norms; first x norms
        dmaj_load(pt_d, 0, chunks[0][1], pT8s[0], 0)
        dmaj_load(xt_d, 0, min(512, b_rows), xT8all, 0)
        for mt in range(mt_n):
            norm_prep(p_d, mt * P, p_norm_sink(mt))
        for ci in range(1, n_ck):
            coff, w = chunks[ci]
            dmaj_load(pt_d, coff, w, pT8s[ci], 0)
        for bt in range(min(PRE, bt_n)):
            norm_prep(x_d, bt * P, x_norm_sink(bt))
        # RP broadcast: rp_d writes sit far ahead of this read on the same
        # SWDGE queue, so the values have landed by the time it drains
        nc.gpsimd.dma_start(RP, rp_d[0:norm_w].partition_broadcast(P))

        for rep in range(repeat):
            for bt in range(bt_n):
                if rep == 0:
                    bg = bt // 4 + 1
                    if bt % 4 == 0 and bg * 512 < b_rows:
                        c0 = bg * 512
                        dmaj_load(
                            xt_d, c0, min(512, b_rows - c0), xT8all, c0
                        )
                    if bt + PRE < bt_n:
                        norm_prep(x_d, (bt + PRE) * P, x_norm_sink(bt + PRE))
                    if p_stat_last and bt == bt_n - 1:
                        nc.gpsimd.dma_start(
                            RX, rx_d[:].partition_broadcast(P)
                        )
                mm_epi(bt)

            if p_stat_last:
                coff, w = chunks[-1]
                n_bg3 = b_rows // 512
                for bg0 in range(0, n_bg3, 2):
                    bgs = range(bg0, min(bg0 + 2, n_bg3))
                    pts3 = {
                        bg: psum.tile([P, 512], f32, tag="ps3", name=f"ps3_{bg}")
                        for bg in bgs
                    }
                    for kp in range(kp_n):
                        for bg in bgs:
                            nc.tensor.matmul(
                                pts3[bg],
                                pT8s[-1][:, 2 * kp : 2 * kp + 2, :],
                                xT8all[
                                    :,
                                    2 * kp : 2 * kp + 2,
                                    bg * 512 : (bg + 1) * 512,
                                ],
                                start=(kp == 0),
                                stop=(kp == kp_n - 1),
                                perf_mode=DR,
                            )
                    for bg in bgs:
                        nc.vector.tensor_mul(
                            pts3[bg], pts3[bg], RX[:, bg * 512 : (bg + 1) * 512]
                        )
                        t3 = opool.tile([P, 512], bf16, tag="t3", name="t3")
                        nc.scalar.activation(
                            t3, pts3[bg], AF.Sqrt, bias=two_s2_b, scale=rp12s
                        )
                        nc.vector.tensor_scalar_mul(t3, t3, -1.0)
                        # transpose [m, 512b] -> [b_inner, 4 b_outer, m]
                        t3T = npool.tile([P, 4, P], bf16, tag="t3T", name="t3T")
                        nc.sync.dma_start(t3T, t3, transpose=True)
                        for bo in range(4):
                            b0 = bg * 512 + bo * P
                            nc.sync.dma_start(
                                o_d[b0 : b0 + P, coff : coff + w], t3T[:, bo, :]
                            )

    nc.compile()
    return nc


LAST_RESULT = None


def _run(nc, in_maps, core_ids):
    from concourse import bass_utils

    global LAST_RESULT
    trace = bool(int(os.environ.get("ISOMAX_TRACE", "0")))
    LAST_RESULT = bass_utils.run_bass_kernel_spmd(
        nc, in_maps, core_ids=core_ids, trace=trace
    )
    return LAST_RESULT.results


def kernel(x, prototypes, distance_scale):
    import ml_dtypes

    bf16 = ml_dtypes.bfloat16
    x = np.asarray(x, dtype=np.float32)
    p = np.asarray(prototypes, dtype=np.float32)
    s_abs = float(abs(np.asarray(distance_scale).reshape(-1)[0].item()))
    m, d = p.shape
    assert (m, d) == (M_FULL, D) and x.shape == (B, D)

    key = ("full", s_abs)
    if key not in _cache:
        _cache[key] = _build(s_abs)
    nc = _cache[key]

    x_bf = np.ascontiguousarray(x.astype(bf16))
    xt_bf = np.ascontiguousarray(x_bf.T)
    p_pad = np.zeros((N_CORES * MC, D), bf16)
    p_pad[:m] = p.astype(bf16)
    in_maps = []
    for i in range(N_CORES):
        pc = np.ascontiguousarray(p_pad[i * MC : (i + 1) * MC])
        in_maps.append(
            {"x": x_bf, "xt": xt_bf, "p": pc, "pt": np.ascontiguousarray(pc.T)}
        )
    results = _run(nc, in_maps, list(range(N_CORES)))
    out = np.concatenate(
        [results[i]["o"].astype(np.float32) for i in range(N_CORES)], axis=1
    )
    return np.ascontiguousarray(out[:, :m])


# revision 4
# speedup vs baseline: 1.3703x; 1.0685x over previous
"""IsoMax pairwise-distance kernel for 8 TRN2 NeuronCores (fp8 DoubleRow, v6).

Math:  out[b,m] = -|s| * sqrt(max(||xn_b||^2 + ||pn_m||^2 - 2*xn_b.pn_m, 0))
with xn/pn L2-normalized rows of x [4096,2048] and prototypes [12893,2048].
Since xn,pn are unit vectors this is -|s|*sqrt(2 - 2*cos(x_b, p_m)).

The PE computes Graw = x_fp8 @ p_fp8^T (RAW values quantized to fp8e4 —
they are O(1) so no scaling/subnormal issues) with perf_mode=DoubleRow.
Normalization happens entirely in the epilogue:
    Gn = Graw * RP          (DVE; RP = 1/||p_m|| broadcast along partitions)
    t  = sqrt(svec_b * Gn + 2s^2)   (ACT; svec_b = -2s^2/||x_b|| per-partition)
    out = -t                (DVE negate), stored bf16, upcast on host.

Earlier revisions transposed x/p on device; the DMA-xbar transposes
serialize (~5-7us round-trip each) and dominated the span. Here the HOST
stages both operands twice: d-major (xT [D,B], pT [D,MC] — fed straight to
fp8 casts for the matmul, one bulk DMA + one DVE cast per 512-column group)
and row-major (only read by the norm pipeline: square+accum -> sqrt ->
reciprocal, all off the matmul critical path). RP (a per-OUTPUT-COLUMN
vector) is produced by writing the 13 per-m-tile reciprocal-norm vectors to
a DRAM scratch buffer and re-reading them with a 0-stride partition
broadcast AP on the same SWDGE queue (the writes sit hundreds of queue
slots ahead of the read).

The last 128-wide psum chunk would be LDWEIGHTS-bound x-stationary (213ns
weight load vs 56ns matmul), so it makes PROTOTYPES stationary and streams
512 b-columns per matmul into [m, b] psum tiles; its per-m factor rides the
ACT scale AP and the per-b factor (RX = 1/||x_b||) is broadcast like RP.
The [m, b] outputs are stored via a DMA xbar transpose (only 8 of them).

Sharding: prototypes split across the 8 cores (output columns), x replicated.
M=12893 padded to 13312 = 8*1664 (zero rows -> harmless, sliced off on host).
"""

import os
import sys

sys.path.insert(0, "/opt/trn_rl_repo")

import numpy as np

B = 4096
D = 2048
M_FULL = 12893
N_CORES = 8
MC = 1664  # per-core prototype rows (13*128); 8*1664 = 13312 >= 12893
P = 128
KT = D // P  # 16 contraction chunks
BT = B // P  # 32 b-tiles

_cache = {}


def _build(s_abs: float, b_rows: int = B, mc: int = MC, repeat: int = 1):
    import concourse.bass as bass  # noqa: F401
    import concourse.mybir as mybir
    import concourse.tile as tile
    from concourse import bacc
    from contextlib import ExitStack

    f32 = mybir.dt.float32
    bf16 = mybir.dt.bfloat16
    fp8 = mybir.dt.float8e4
    AF = mybir.ActivationFunctionType
    DR = mybir.MatmulPerfMode.DoubleRow
    kt = D // P
    kp_n = kt // 2
    mt_n = mc // P
    bt_n = b_rows // P
    two_s2 = 2.0 * s_abs * s_abs

    # psum chunks over mc columns; 512-wide, aligned with prototype m-tiles
    chunks = []
    off = 0
    while off < mc:
        w = min(512, mc - off)
        chunks.append((off, w))
        off += w
    n_ck = len(chunks)
    p_stat_last = chunks[-1][1] == P and b_rows % 512 == 0
    n_norm_ck = n_ck - 1 if p_stat_last else n_ck
    norm_w = chunks[n_norm_ck - 1][0] + chunks[n_norm_ck - 1][1]
    bg_n = (b_rows + 511) // 512

    nc = bacc.Bacc(None, target_bir_lowering=False)
    x_d = nc.dram_tensor("x", [b_rows, D], bf16, kind="ExternalInput")
    p_d = nc.dram_tensor("p", [mc, D], bf16, kind="ExternalInput")
    xt_d = nc.dram_tensor("xt", [D, b_rows], bf16, kind="ExternalInput")
    pt_d = nc.dram_tensor("pt", [D, mc], bf16, kind="ExternalInput")
    o_d = nc.dram_tensor("o", [b_rows, mc], bf16, kind="ExternalOutput")
    rp_d = nc.dram_tensor("rp_scratch", [mc], f32, kind="Internal")
    rx_d = nc.dram_tensor("rx_scratch", [b_rows], f32, kind="Internal")

    with ExitStack() as ctx:
        tc = ctx.enter_context(tile.TileContext(nc))
        persist = ctx.enter_context(tc.tile_pool(name="persist", bufs=1))
        dpool = ctx.enter_context(tc.tile_pool(name="dpool", bufs=2))
        npool = ctx.enter_context(tc.tile_pool(name="npool", bufs=4))
        sq = ctx.enter_context(tc.tile_pool(name="sq", bufs=2))
        small = ctx.enter_context(tc.tile_pool(name="small", bufs=8))
        opool = ctx.enter_context(tc.tile_pool(name="opool", bufs=4))
        psum = ctx.enter_context(tc.tile_pool(name="psum", bufs=2, space="PSUM"))

        # fp8 d-major operands: [d_inner, k_chunk, col]
        pT8s = [
            persist.tile([P, kt, w], fp8, tag=f"pT8_{ci}", name=f"pT8_{ci}")
            for ci, (_o, w) in enumerate(chunks)
        ]
        xT8all = persist.tile([P, kt, b_rows], fp8, tag="xT8all")
        # epilogue factors
        svec_all = persist.tile([P, bt_n], f32, tag="svec_all")
        RP = persist.tile([P, mc], bf16, tag="RP")  # 32/||p_m|| bcast
        if p_stat_last:
            RX = persist.tile([P, b_rows], bf16, tag="RX")  # 1/||x_b|| bcast

        tiny_b = persist.tile([P, 1], f32, tag="tiny_b")
        nc.vector.memset(tiny_b, 1e-30)
        two_s2_b = persist.tile([P, 1], f32, tag="two_s2_b")
        nc.vector.memset(two_s2_b, two_s2)

        def dmaj_load(src_t, c0, w, dst8, dc0, scale_rp=False):
            # bulk d-major block [128, kt, w] -> fp8 into dst8[:, :, dc0:+w];
            # for prototypes the 32/||p_m|| factor is folded into the cast
            # (free-dim broadcast of RP over partitions' k dim)
            blk = dpool.tile([P, kt, 512], bf16, tag="blk", name="blk")[:, :, :w]
            src = src_t[:, c0 : c0 + w].rearrange(
                "(k p) c -> p k c", p=P
            )
            nc.gpsimd.dma_start(blk, src)
            if scale_rp:
                rp_b = RP[:, c0 : c0 + w].unsqueeze(1).broadcast_to([P, kt, w])
                nc.vector.tensor_mul(dst8[:, :, dc0 : dc0 + w], blk, rp_b)
            else:
                nc.vector.tensor_copy(dst8[:, :, dc0 : dc0 + w], blk)

        def norm_prep(src_d, row0, sink, scale=1.0):
            # row-major 128 rows -> scale/||row|| [128,1], handed to sink
            r_f = npool.tile([P, D], bf16, tag="r_f", name="r_f")
            nc.sync.dma_start(r_f, src_d[row0 : row0 + P, :])
            rsq = sq.tile([P, D], bf16, tag="sq", name="rsq")
            ss = small.tile([P, 1], f32, tag="ss", name="ss")
            nc.scalar.activation(rsq, r_f, AF.Square, accum_out=ss)
            rnorm = small.tile([P, 1], f32, tag="nrm", name="rnorm")
            nc.scalar.activation(
                rnorm, ss, AF.Sqrt, bias=tiny_b, scale=1.0 / (scale * scale)
            )
            rr = small.tile([P, 1], f32, tag="rr", name="rr")
            nc.vector.reciprocal(rr, rnorm)
            sink(rr)

        def p_norm_sink(mt):
            def sink(rr):
                # stash 32/||p|| to DRAM for the RP broadcast read
                nc.gpsimd.dma_start(rp_d[mt * P : (mt + 1) * P], rr)
            return sink

        def x_norm_sink(bt):
            def sink(rr):
                # -2s^2/(32*||x_b||): G = 32*cos*||x_b||
                nc.vector.tensor_scalar_mul(
                    svec_all[:, bt : bt + 1], rr, -two_s2 / 32.0
                )
                if p_stat_last:
                    nc.gpsimd.dma_start(rx_d[bt * P : (bt + 1) * P], rr)
            return sink

        def mm_epi(bt):
            pt = psum.tile([P, norm_w], f32, tag="psA", name="pt")
            for ci in range(n_norm_ck):
                coff, w = chunks[ci]
                for kp in range(kp_n):
                    nc.tensor.matmul(
                        pt[:, coff : coff + w],
                        xT8all[:, 2 * kp : 2 * kp + 2, bt * P : (bt + 1) * P],
                        pT8s[ci][:, 2 * kp : 2 * kp + 2, :],
                        start=(kp == 0),
                        stop=(kp == kp_n - 1),
                        perf_mode=DR,
                    )
            t_c = opool.tile([P, norm_w], bf16, tag="t_c", name="t_c")
            nc.scalar.activation(
                t_c, pt, AF.Sqrt, bias=two_s2_b,
                scale=svec_all[:, bt : bt + 1],
            )
            nc.vector.tensor_scalar_mul(t_c, t_c, -1.0)
            nc.sync.dma_start(o_d[bt * P : (bt + 1) * P, 0:norm_w], t_c)

        PRE = 6
        # prologue: first x b-groups (independent of RP), all p norms (their
        # rp writes land early), first x norms, RP broadcast (several bulk
        # loads behind the last rp write on the same SWDGE queue), then the
        # prototype d-major loads whose casts consume RP
        dmaj_load(xt_d, 0, min(512, b_rows), xT8all, 0)
        for mt in range(mt_n):
            norm_prep(p_d, mt * P, p_norm_sink(mt), scale=32.0)
        if b_rows > 512:
            dmaj_load(xt_d, 512, min(512, b_rows - 512), xT8all, 512)
        for bt in range(min(PRE, bt_n)):
            norm_prep(x_d, bt * P, x_norm_sink(bt))
        nc.gpsimd.dma_start(RP, rp_d[0:mc].partition_broadcast(P))
        for ci in range(n_ck):
            coff, w = chunks[ci]
            dmaj_load(pt_d, coff, w, pT8s[ci], 0, scale_rp=True)

        for rep in range(repeat):
            for bt in range(bt_n):
                if rep == 0:
                    bg = bt // 4 + 2
                    if bt % 4 == 0 and bg * 512 < b_rows:
                        c0 = bg * 512
                        dmaj_load(
                            xt_d, c0, min(512, b_rows - c0), xT8all, c0
                        )
                    if bt + PRE < bt_n:
                        norm_prep(x_d, (bt + PRE) * P, x_norm_sink(bt + PRE))
                    if p_stat_last and bt == bt_n - 1:
                        nc.gpsimd.dma_start(
                            RX, rx_d[:].partition_broadcast(P)
                        )
                mm_epi(bt)

            if p_stat_last:
                coff, w = chunks[-1]
                n_bg3 = b_rows // 512
                for bg0 in range(0, n_bg3, 2):
                    bgs = range(bg0, min(bg0 + 2, n_bg3))
                    pts3 = {
                        bg: psum.tile([P, 512], f32, tag="ps3", name=f"ps3_{bg}")
                        for bg in bgs
                    }
                    for kp in range(kp_n):
                        for bg in bgs:
                            nc.tensor.matmul(
                                pts3[bg],
                                pT8s[-1][:, 2 * kp : 2 * kp + 2, :],
                                xT8all[
                                    :,
                                    2 * kp : 2 * kp + 2,
                                    bg * 512 : (bg + 1) * 512,
                                ],
                                start=(kp == 0),
                                stop=(kp == kp_n - 1),
                                perf_mode=DR,
                            )
                    for bg in bgs:
                        nc.vector.tensor_mul(
                            pts3[bg], pts3[bg], RX[:, bg * 512 : (bg + 1) * 512]
                        )
                        t3 = opool.tile([P, 512], bf16, tag="t3", name="t3")
                        nc.scalar.activation(
                            t3, pts3[bg], AF.Sqrt, bias=two_s2_b,
                            scale=-two_s2 / 32.0,
                        )
                        nc.vector.tensor_scalar_mul(t3, t3, -1.0)
                        # transpose [m, 512b] -> [b_inner, 4 b_outer, m]
                        t3T = npool.tile([P, 4, P], bf16, tag="t3T", name="t3T")
                        nc.sync.dma_start(t3T, t3, transpose=True)
                        for bo in range(4):
                            b0 = bg * 512 + bo * P
                            nc.sync.dma_start(
                                o_d[b0 : b0 + P, coff : coff + w], t3T[:, bo, :]
                            )

    nc.compile()
    return nc


LAST_RESULT = None


def _run(nc, in_maps, core_ids):
    from concourse import bass_utils

    global LAST_RESULT
    trace = bool(int(os.environ.get("ISOMAX_TRACE", "0")))
    LAST_RESULT = bass_utils.run_bass_kernel_spmd(
        nc, in_maps, core_ids=core_ids, trace=trace
    )
    return LAST_RESULT.results


def kernel(x, prototypes, distance_scale):
    import ml_dtypes

    bf16 = ml_dtypes.bfloat16
    x = np.asarray(x, dtype=np.float32)
    p = np.asarray(prototypes, dtype=np.float32)
    s_abs = float(abs(np.asarray(distance_scale).reshape(-1)[0].item()))
    m, d = p.shape
    assert (m, d) == (M_FULL, D) and x.shape == (B, D)

    key = ("full", s_abs)
    if key not in _cache:
        _cache[key] = _build(s_abs)
    nc = _cache[key]

    x_bf = np.ascontiguousarray(x.astype(bf16))
    xt_bf = np.ascontiguousarray(x_bf.T)
    p_pad = np.zeros((N_CORES * MC, D), bf16)
    p_pad[:m] = p.astype(bf16)
    in_maps = []
    for i in range(N_CORES):
        pc = np.ascontiguousarray(p_pad[i * MC : (i + 1) * MC])
        in_maps.append(
            {"x": x_bf, "xt": xt_bf, "p": pc, "pt": np.ascontiguousarray(pc.T)}
        )
    results = _run(nc, in_maps, list(range(N_CORES)))
    out = np.concatenate(
        [results[i]["o"].astype(np.float32) for i in range(N_CORES)], axis=1
    )
    return np.ascontiguousarray(out[:, :m])
